# revision 2
# baseline (speedup 1.0000x reference)
"""Trainium2 Bass kernel (v6) for the Dale CB-cell step.

Pipelined, feature-major, half-packed layout; zero on-device transposes.

Host prep builds F[64, B]: rows 0-47 = hidden.T, rows 48-55 =
logit((x-lo)/span) (so one sigmoid pass reproduces the affine x), row 56
= 30 (sigmoid -> 1, the bias row), rows 57-63 = 0. Each C-column tick is
packed [128, C/2] (partitions 0-63 = first half of the columns, 64-127 =
second half) and stored as a contiguous [128, C/2] bf16 DRAM block —
measured: contiguous bf16 block DMA runs ~300+ GB/s on this stack while
row-slices of a wide 2D tensor run ~26 GB/s.

All weights fold into one [64, 128] stationary block: cols 0-47 = z half
(Ksp/P_z/b_z), cols 64-111 = u half (DT*W/DT*P_masked/DT*b_v), rest
zero, so each [64,128]x[64,512] matmul writes z to PSUM partitions 0-47
and u to 64-111 (both 32-aligned, the hardware requirement for reads).

Measured engine facts that shape the op mix:
  - PSUM reads cost per-lane: evict full [128,2048] tiles (~2.1us) not
    [48,2048] pieces (2.5-4us each).
  - SBUF->SBUF bf16 copies run ~6 elem/cyc/lane; partition-base shifts
    are free -> repack z/u into 128-lane tiles with [48, C/2] copies.
  - Two SBUF operands of one op must share a base partition; PSUM+SBUF
    may differ. All epilogue ops are arranged 128-lane and base-aligned.
  - Loop-body instructions cost ~1us amortized; straight-line (unrolled)
    instructions and per-loop-instance body fetch cost ~40-100us each,
    so everything hot lives in ONE For_i body and the body is kept small
    (C=4096 -> ~42 instructions for the 2-tick body).

4-stage software pipeline inside one tc.For_i (2-tick unrolled body,
tile slot = macro parity; every read is at pipeline distance <= 2):
  S3b (macro T-3): vnew = (q * -DT) + upkv  [one fused stt]; DMA store
  S3a (macro T-2): repack u->up, z->zA; zB = sigmoid(zA);
                   q = zB*hv -> zA; upkv = up + hv -> up
                   (vnew = v - DT*sigmoid(zpre)*v + u, associativity
                    chosen so S3b needs no tile older than distance 1)
  S2  (macro T-1): per 2048-col group: 4 matmuls -> ps[g%2], then one
                   full [128,2048] DVE evict into zA/zB
  S01 (macro T):   DMA load hv; ACT sigmoid -> rb; DVE shift rb[64:]->rbs
                   (matmul rhs must sit at partition base 0)
DRAM arrays are padded 3 blocks before / 5 after the data so pipeline
edge ticks compute garbage into the pads instead of needing straight-
line prologue code. Stages touch different macros, so DMA/ACT/PE/DVE
overlap; measured steady state ~19us per 4096-col tick per core.
"""

import sys

if "/opt/trn_rl_repo" not in sys.path:
    sys.path.insert(0, "/opt/trn_rl_repo")

import numpy as np

H = 48
IN = 8
DT = 0.1
B = 1048576
N_CORES = 8
B_CORE = B // N_CORES          # 131072
C_DEF = 4096                   # batch cols per pipeline tick
PADL = 3                       # pad macro-blocks before data

_NC_CACHE = {}


def _softplus64(x):
    x = x.astype(np.float64)
    return np.log1p(np.exp(-np.abs(x))) + np.maximum(x, 0.0)


def _build_wblk128(P, b_v, K, C, P_z, b_z, e_e, e_i, lo, hi):
    """Fold all weights into the (64, 128) matmul lhsT block (float64)."""
    Ksp = _softplus64(K)
    Csp = _softplus64(C)
    S = Ksp + Csp
    e_e = float(np.asarray(e_e).reshape(-1)[0])
    e_i = float(np.asarray(e_i).reshape(-1)[0])
    W_E = np.maximum(e_e * S[:, : H // 2], 0.0)
    W_I = -np.maximum(-(e_i * S[:, H // 2 :]), 0.0)
    W = np.concatenate([W_E, W_I], axis=1)          # (H, H)
    rows = np.arange(H)
    keep = ~(((rows >= H // 4) & (rows < H // 2)) | (rows >= 3 * H // 4))
    P_masked = P.astype(np.float64) * keep[:, None]
    P_z = P_z.astype(np.float64)
    span = hi - lo

    blk = np.zeros((64, 128), np.float64)
    blk[0:H, 0:H] = Ksp.T
    blk[H : H + IN, 0:H] = (span * P_z).T
    blk[H + IN, 0:H] = b_z.astype(np.float64).reshape(-1) + lo * P_z.sum(axis=1)
    blk[0:H, 64 : 64 + H] = (DT * W).T
    blk[H : H + IN, 64 : 64 + H] = (DT * span * P_masked).T
    blk[H + IN, 64 : 64 + H] = DT * (
        b_v.astype(np.float64).reshape(-1) + lo * P_masked.sum(axis=1)
    )
    return blk


def _build_nc_v6(b_core, reps=1, bench=False, c_tick=C_DEF, dup=False):
    """reps>1 repeats the pass for benching: dup=False nests the tick loop
    in an outer For_i (body fetched once -> measures steady state);
    dup=True duplicates the loop per rep (each instance refetches ->
    measures the cold-pass cost, the baseline-compatible metric)."""
    import concourse.bacc as bacc
    import concourse.mybir as mybir
    import concourse.tile as tile
    from concourse.bass import ds

    F32 = mybir.dt.float32
    BF16 = mybir.dt.bfloat16
    SIG = mybir.ActivationFunctionType.Sigmoid
    MULT = mybir.AluOpType.mult
    ADD = mybir.AluOpType.add

    MH = c_tick // 2                 # packed cols per tick
    NG = c_tick // 2048              # psum groups per tick (>= 2)
    assert NG >= 2 and NG % 2 == 0
    n = b_core // c_tick             # real macros
    ni = n + 4                       # inner ticks (even)
    nblk = n + PADL + 5              # DRAM macro-blocks incl. pads

    nc = bacc.Bacc("TRN2", target_bir_lowering=False, debug=False)
    big = "Internal" if bench else None
    hidT = nc.dram_tensor("hidT", [nblk * 128, MH], BF16,
                          kind=big or "ExternalInput")
    wdram = nc.dram_tensor("wdram", [64, 128], BF16, kind="ExternalInput")
    outT = nc.dram_tensor("outT", [nblk * 128, MH], BF16,
                          kind=big or "ExternalOutput")
    dbg = (nc.dram_tensor("dbg", [128, 64], F32, kind="ExternalOutput")
           if bench else None)

    with tile.TileContext(nc) as tc:
        with (
            tc.tile_pool(name="sb", bufs=1) as sb,
            tc.tile_pool(name="psum", bufs=1, space="PSUM") as pp,
        ):
            w_sb = sb.tile([64, 128], BF16)
            nc.sync.dma_start(w_sb[:], wdram[:])

            hv = [sb.tile([128, MH], BF16, name=f"hv{s}") for s in range(2)]
            rb = [sb.tile([128, MH], BF16, name=f"rb{s}") for s in range(2)]
            rbs = [sb.tile([64, MH], BF16, name=f"rbs{s}") for s in range(2)]
            zA = [sb.tile([128, MH], BF16, name=f"zA{s}") for s in range(2)]
            zB = [sb.tile([128, MH], BF16, name=f"zB{s}") for s in range(2)]
            up = [sb.tile([128, MH], BF16, name=f"up{s}") for s in range(2)]
            ps = [pp.tile([128, 2048], F32, name=f"ps{s}") for s in range(2)]

            # Seed tiles that pipeline edge ticks read before their first
            # in-loop write — an unwritten-tile read wedges the device.
            for t_ in hv + rb + rbs + zA + zB + up:
                nc.gpsimd.memset(t_[:], 0.0)

            def tick(iv, sl):
                o = 1 - sl
                # S3b: macro T-3 (slot o)
                nc.vector.scalar_tensor_tensor(
                    up[o][:], zA[o][:], -DT, up[o][:], MULT, ADD)
                nc.sync.dma_start(outT[ds(iv * 128, 128), :], up[o][:])
                # S3a: macro T-2 (slot sl)
                nc.vector.tensor_copy(up[sl][0:H, :], zA[sl][64 : 64 + H, :])
                nc.vector.tensor_copy(
                    up[sl][64 : 64 + H, :], zB[sl][64 : 64 + H, :])
                nc.vector.tensor_copy(zA[sl][64 : 64 + H, :], zB[sl][0:H, :])
                nc.scalar.activation(zB[sl][:], zA[sl][:], SIG)
                nc.vector.tensor_tensor(zA[sl][:], zB[sl][:], hv[sl][:], op=MULT)
                nc.vector.tensor_tensor(up[sl][:], up[sl][:], hv[sl][:], op=ADD)
                # S2: macro T-1 (slot o)
                for g in range(NG):
                    src = rb[o] if g < NG // 2 else rbs[o]
                    c0 = (g % (NG // 2)) * 2048
                    pst = ps[g % 2]
                    for s in range(4):
                        nc.tensor.matmul(
                            pst[:, 512 * s : 512 * s + 512],
                            w_sb[:], src[0:64, c0 + 512 * s : c0 + 512 * s + 512],
                            start=True, stop=True)
                    dst = zA[o] if g < NG // 2 else zB[o]
                    nc.vector.tensor_copy(dst[:, c0 : c0 + 2048], pst[:])
                # S01: macro T (slot sl)
                nc.sync.dma_start(hv[sl][:], hidT[ds((iv + PADL) * 128, 128), :])
                nc.scalar.activation(rb[sl][:], hv[sl][:], SIG)
                nc.vector.tensor_copy(rbs[sl][:], rb[sl][64:128, :])

            def inner():
                with tc.For_i(0, ni, 2) as iv:
                    tick(iv, 0)
                    tick(iv + 1, 1)

            if dup:
                for _ in range(reps):
                    inner()
            elif reps == 1:
                inner()
            else:
                with tc.For_i(0, reps, 1):
                    inner()

            if bench:
                dbg_t = sb.tile([128, 64], F32, name="dbg_t")
                nc.gpsimd.memset(dbg_t[:], 0.0)
                nc.sync.dma_start(dbg[:], dbg_t[:])

    nc.compile()
    return nc


def get_nc_v6(b_core=B_CORE, reps=1, bench=False, c_tick=C_DEF, dup=False):
    key = ("v6", b_core, reps, bench, c_tick, dup)
    if key not in _NC_CACHE:
        _NC_CACHE[key] = _build_nc_v6(b_core, reps, bench, c_tick, dup)
    return _NC_CACHE[key]


def _pack_blocks(Fk, n, MH):
    """(64, n*C) feature-major -> (n*128, MH) half-packed block layout."""
    return (
        Fk.reshape(64, n, 2, MH)
        .transpose(1, 2, 0, 3)
        .reshape(n * 128, MH)
    )


def _unpack_blocks(Od, n, MH):
    return (
        Od.reshape(n, 2, 64, MH)
        .transpose(2, 0, 1, 3)
        .reshape(64, n * 2 * MH)
    )


def prepare_inputs_v6(hidden, x, P, b_v, K, C, P_z, b_z, e_e, e_i,
                      n_cores=N_CORES, c_tick=C_DEF):
    import ml_dtypes

    bf16 = ml_dtypes.bfloat16
    hidden = np.asarray(hidden, np.float32)
    x = np.asarray(x, np.float64)                  # (IN, B)
    lo = float(x.min()) - 1e-3
    hi = float(x.max()) + 1e-3
    xt = (x - lo) / (hi - lo)
    t = np.log(xt / (1.0 - xt)).astype(np.float32)  # logit, (IN, B)

    blk = _build_wblk128(
        np.asarray(P), np.asarray(b_v), np.asarray(K), np.asarray(C),
        np.asarray(P_z), np.asarray(b_z), np.asarray(e_e), np.asarray(e_i),
        lo, hi,
    ).astype(bf16)

    btot = hidden.shape[0]
    F = np.zeros((64, btot), bf16)
    F[0:H, :] = hidden.T
    F[H : H + IN, :] = t
    F[H + IN, :] = 30.0

    MH = c_tick // 2
    b_core = btot // n_cores
    n = b_core // c_tick
    nblk = n + PADL + 5
    in_maps = []
    for k in range(n_cores):
        Fk = F[:, k * b_core : (k + 1) * b_core]
        hid_dev = np.zeros((nblk * 128, MH), bf16)
        hid_dev[PADL * 128 : (PADL + n) * 128, :] = _pack_blocks(Fk, n, MH)
        in_maps.append({"hidT": hid_dev, "wdram": blk})
    return in_maps


def kernel(hidden, x, P, b_v, K, C, P_z, b_z, e_e, e_i):
    from concourse.bass_utils import run_bass_kernel_spmd

    c_tick = C_DEF
    MH = c_tick // 2
    nc = get_nc_v6(B_CORE, c_tick=c_tick)
    in_maps = prepare_inputs_v6(hidden, x, P, b_v, K, C, P_z, b_z, e_e, e_i,
                                c_tick=c_tick)
    res = run_bass_kernel_spmd(nc, in_maps, list(range(N_CORES)))
    n = B_CORE // c_tick
    outs = []
    for r in res.results:
        Od = np.asarray(r["outT"])[PADL * 128 : (PADL + n) * 128, :]
        outs.append(_unpack_blocks(Od, n, MH)[0:H, :].T)
    return np.concatenate(outs, axis=0).astype(np.float32)
